# revision 4
# baseline (speedup 1.0000x reference)
"""Trainium2 Bass kernel v6 for the pre-norm transformer block.

Shapes (hardcoded): x [B=4, N=2048, C=384], HEADS=6, HEAD_DIM=64, HID=1536.
Sharding: 8 cores = (batch, query-half), no cross-core communication.

v6 (v5 = 266us, baseline v3 = 305us). ScalarE is the floor engine
(96 exps ~1.11us + 24 tanhs ~0.82us ~= 127us); everything else hides under
it. v6 closes v5's three holes:
  - startup (was 56us to first exp): x is bf16 end-to-end (token-major xq is
    both the LN1-stats source and the residual), identities/memsets emitted
    before the weight-DMA flood, one bf16 stats bounce, 3-iter Newton rsqrt.
  - pair-boundary exp gaps (was ~35us): PV runs 4 kc behind S^T/exp, each
    pair's last PV steps + normalization are emitted inside the NEXT pair's
    kc loop (finisher), popped work uses psumM halves so the s_ps double
    buffer never waits on popped chunks, fc2 pops are paced 1-per-2-kc.
  - tail (was 52us): batched z2 transposes, alternating psum slots for
    fc1/fc2, shorter Newton.
"""

import numpy as np
import ml_dtypes

B, N, C = 4, 2048, 384
HEADS, HEAD_DIM = 6, 64
HID = 1536
EPS = 1e-5
NCORES = 8
T = N
TQ = N // 2
CC = C // 128
NT = T // 128
NTQ = TQ // 128
MH = HID // 128
QH = 512
GA = 0.851

_COMPILED = None


def build_nc():
    import concourse.bass as bass
    import concourse.tile as tile
    from concourse import bacc, mybir
    from concourse.masks import make_identity

    f32 = mybir.dt.float32
    bf16 = mybir.dt.bfloat16
    AF = mybir.ActivationFunctionType
    ALU = mybir.AluOpType

    nc = bacc.Bacc("TRN2", target_bir_lowering=False, debug=False,
                   num_devices=NCORES)

    from concourse.bacc import get_activation_tables
    tabs = get_activation_tables(nc.m.arch)
    for name, s in tabs.items():
        if name != "exp_and_others":
            s.discard(AF.Exp)
            s.discard(AF.Tanh)

    # all big inputs host-prearranged partition-major: one large contiguous
    # descriptor per partition per DMA
    xq_d = nc.dram_tensor("xq", [128, NT, C], bf16, kind="ExternalInput").ap()
    xt_d = nc.dram_tensor("xt", [128, CC, T], bf16, kind="ExternalInput").ap()
    wqk_d = nc.dram_tensor("wqk", [128, CC, 2 * C], bf16,
                           kind="ExternalInput").ap()
    rqk_d = nc.dram_tensor("rqk", [2, 2 * C], bf16, kind="ExternalInput").ap()
    wv_d = nc.dram_tensor("wv", [128, CC, C], bf16, kind="ExternalInput").ap()
    svbv_d = nc.dram_tensor("svbv", [2, C], bf16, kind="ExternalInput").ap()
    wp_d = nc.dram_tensor("wp", [128, CC, C], bf16, kind="ExternalInput").ap()
    bp_d = nc.dram_tensor("bp", [1, C], bf16, kind="ExternalInput").ap()
    w1_d = nc.dram_tensor("w1", [128, CC, HID], bf16,
                          kind="ExternalInput").ap()
    b1_d = nc.dram_tensor("b1", [128, 2, MH], f32, kind="ExternalInput").ap()
    w2_d = nc.dram_tensor("w2", [128, MH, C], bf16, kind="ExternalInput").ap()
    b2_d = nc.dram_tensor("b2", [1, C], bf16, kind="ExternalInput").ap()
    out_d = nc.dram_tensor("out", [TQ, C], f32, kind="ExternalOutput").ap()

    def bcast_load(engine, dst, src_ap, parts=128):
        engine.dma_start(dst, bass.AP(tensor=src_ap.tensor,
                                      offset=src_ap.offset,
                                      ap=[[0, parts]] + list(src_ap.ap)))

    with tile.TileContext(nc) as tc:
        with (
            tc.tile_pool(name="singles", bufs=1) as singles,
            tc.tile_pool(name="work", bufs=4) as work,
            tc.tile_pool(name="stats", bufs=4) as stats,
            tc.tile_pool(name="attn", bufs=13) as attn_pool,
            tc.tile_pool(name="psumA", bufs=2, space="PSUM") as psumA,
            tc.tile_pool(name="psumPV", bufs=2, space="PSUM") as psumPV,
            tc.tile_pool(name="psumM", bufs=2, space="PSUM") as psumM,
            tc.tile_pool(name="dram", bufs=2, space="DRAM") as dram,
        ):
            # ---- PE warmup (clock-gate / p-state ramp while DMAs land) ----
            warm_w = singles.tile([128, 128], bf16, tag="warm_w")
            warm_x = singles.tile([128, 512], bf16, tag="warm_x")
            nc.vector.memset(warm_w, 0.0)
            nc.vector.memset(warm_x, 0.0)
            for wi in range(12):
                wps = psumA.tile([128, 512], f32, tag="A", name=f"warm{wi}")
                nc.tensor.matmul(wps, warm_w, warm_x, start=True, stop=True)

            # identities/ones early: gpsimd must run these before its DMA
            # issue backlog builds up
            onesw = singles.tile([1, 128], bf16, tag="onesw")
            nc.vector.memset(onesw, 1.0)
            onesf = singles.tile([1, HEAD_DIM], f32, tag="onesf")
            nc.vector.memset(onesf, 1.0)
            identb = singles.tile([128, 128], bf16, tag="identb")
            make_identity(nc, identb)
            identf = singles.tile([128, 128], f32, tag="identf")
            make_identity(nc, identf)

            # ---- DMAs. gpsimd: QKV-critical first, late weights after.
            # sync: xq + kv stat chunks + bounce + outputs. scalar: none. ----
            wqk = singles.tile([128, CC, 2 * C], bf16, tag="wqk")
            nc.gpsimd.dma_start(wqk, wqk_d)
            xt3 = singles.tile([128, CC, T], bf16, tag="big12")
            for s4 in range(4):
                nc.gpsimd.dma_start(xt3[:, :, s4 * 512:(s4 + 1) * 512],
                                    xt_d[:, :, s4 * 512:(s4 + 1) * 512])
            wv = singles.tile([128, CC, C], bf16, tag="wv")
            nc.gpsimd.dma_start(wv, wv_d)
            rqk = singles.tile([2, 2 * C], bf16, tag="rqk")
            nc.gpsimd.dma_start(rqk, rqk_d)
            svbv = singles.tile([2, C], bf16, tag="svbv")
            nc.gpsimd.dma_start(svbv, svbv_d)

            xq = singles.tile([128, NT, C], bf16, tag="xq")
            for g in range(4):
                nc.sync.dma_start(xq[:, g * 4:(g + 1) * 4, :],
                                  xq_d[:, g * 4:(g + 1) * 4, :])

            # LN1 stats on DVE
            mv1 = singles.tile([128, NT, 2], f32, tag="mv1")
            for i in range(NT):
                st = stats.tile([128, 6], f32, tag="bnst")
                nc.vector.bn_stats(st, xq[:, i, :])
                nc.vector.bn_aggr(mv1[:, i, :], st)

            # late loads (gpsimd)
            wp = singles.tile([128, CC, C], bf16, tag="wp")
            nc.gpsimd.dma_start(wp, wp_d)
            bp_row = singles.tile([1, C], bf16, tag="bp_row")
            nc.gpsimd.dma_start(bp_row, bp_d)
            w1 = singles.tile([128, CC, HID], bf16, tag="w1")
            nc.gpsimd.dma_start(w1, w1_d)
            b1c = singles.tile([128, 2, MH], f32, tag="b1c")
            nc.gpsimd.dma_start(b1c, b1_d)
            w2 = singles.tile([128, MH, C], bf16, tag="w2")
            nc.gpsimd.dma_start(w2, w2_d)
            b2_row = singles.tile([1, C], bf16, tag="b2_row")
            nc.gpsimd.dma_start(b2_row, b2_d)

            # ---- persistent SBUF tensors ----
            qT = singles.tile([128, CC, TQ], bf16, tag="qT")
            kT = singles.tile([128, CC, T], bf16, tag="kT")
            vauge = singles.tile([128, NT, CC, HEAD_DIM + 1], bf16, tag="vauge")
            vaugo = singles.tile([128, NT, CC, 128], bf16, tag="vaugo")
            oT = singles.tile([128, CC, TQ], bf16, tag="oT")
            x2 = singles.tile([128, NTQ, C], f32, tag="x2")
            z2 = singles.tile([128, NTQ, C], bf16, tag="z2")
            z2T = singles.tile([128, CC, TQ], bf16, tag="z2T")
            gT_hi = singles.tile([128, MH // 2, TQ], bf16, tag="gT_hi")
            sB = singles.tile([128, T], bf16, tag="sB")
            miv = singles.tile([2, T], bf16, tag="miv")
            nc.vector.memset(vauge[:, :, :, HEAD_DIM:], 1.0)
            nc.vector.memset(vaugo[:, :, :, 0:HEAD_DIM], 0.0)
            nc.vector.memset(vaugo[:, :, :, 0:1], 1.0)

            def newton_rsqrt(y, var, k, tmp_tag):
                """y[128,k] <- rsqrt(var+EPS), seed 1.0 (LN input var ~= 1)."""
                u = stats.tile([128, k], f32, tag=tmp_tag, name=tmp_tag + "u")
                nc.vector.tensor_scalar_add(u, var, float(EPS))
                nc.vector.memset(y, 1.0)
                for it in range(3):
                    t1 = stats.tile([128, k], f32, tag=tmp_tag + "t",
                                    name=f"{tmp_tag}t{it}", bufs=2)
                    nc.vector.tensor_tensor(t1, y, y, ALU.mult)
                    nc.vector.scalar_tensor_tensor(t1, t1, -0.5, u,
                                                   ALU.mult, ALU.mult)
                    nc.vector.scalar_tensor_tensor(y, t1, 1.5, y,
                                                   ALU.add, ALU.mult)

            # stats pack [128, 4, NT]: rstd | pad | m | invrstd
            stp1 = singles.tile([128, 4, NT], f32, tag="stp1")
            nc.vector.memset(stp1[:, 1, :], 0.0)
            newton_rsqrt(stp1[:, 0, :], mv1[:, :, 1], NT, "n1")
            nc.vector.tensor_copy(stp1[:, 2, :], mv1[:, :, 0])
            sc = stats.tile([128, NT], f32, tag="n1", name="n1v")
            nc.vector.tensor_scalar_add(sc, mv1[:, :, 1], float(EPS))
            nc.vector.tensor_tensor(stp1[:, 3, :], stp1[:, 0, :], sc, ALU.mult)

            # ---- QKV ----
            def qk_mains(m, n2, ps):
                for h2 in range(2):
                    n0 = n2 * 1024 + h2 * 512
                    for c in range(CC):
                        nc.tensor.matmul(
                            ps[:, h2 * 512:(h2 + 1) * 512],
                            wqk[:, c, m * 128:(m + 1) * 128],
                            xt3[:, c, n0:n0 + 512],
                            start=(c == 0), stop=False)

            def qk_finish(m, n2, ps):
                is_q = m < CC
                for h2 in range(2):
                    sl = slice(h2 * 512, (h2 + 1) * 512)
                    n0 = n2 * 1024 + h2 * 512
                    nc.tensor.matmul(ps[:, sl], rqk[:, m * 128:(m + 1) * 128],
                                     miv[:, n0:n0 + 512], start=False,
                                     stop=True)
                dst = (qT[:, m, :] if is_q else
                       kT[:, m - CC, n2 * 1024:(n2 + 1) * 1024])
                nc.vector.tensor_tensor(
                    dst, ps, sB[:, n2 * 1024:(n2 + 1) * 1024], ALU.mult)

            def qk_half(m, n2, h2):
                """512-col QK block on a psumM slot (popped work must not
                touch the s_ps double buffer)."""
                ps = psumM.tile([128, QH], f32, tag="M")
                n0 = n2 * 1024 + h2 * 512
                for c in range(CC):
                    nc.tensor.matmul(ps, wqk[:, c, m * 128:(m + 1) * 128],
                                     xt3[:, c, n0:n0 + 512],
                                     start=(c == 0), stop=False)
                nc.tensor.matmul(ps, rqk[:, m * 128:(m + 1) * 128],
                                 miv[:, n0:n0 + 512], start=False, stop=True)
                dst = (qT[:, m, n0:n0 + 512] if m < CC else
                       kT[:, m - CC, n0:n0 + 512])
                nc.vector.tensor_tensor(dst, ps, sB[:, n0:n0 + 512], ALU.mult)

            def v_chunk(tk):
                ps = psumM.tile([128, C], f32, tag="M")
                for c in range(CC):
                    nc.tensor.matmul(ps, xt3[:, c, tk * 128:(tk + 1) * 128],
                                     wv[:, c, :], start=(c == 0), stop=False)
                nc.tensor.matmul(ps, miv[:, tk * 128:(tk + 1) * 128],
                                 svbv, start=False, stop=True)
                ps_h = ps.rearrange("p (h d) -> p h d", h=HEADS)
                nc.vector.tensor_scalar_mul(
                    vauge[:, tk, :, 0:HEAD_DIM], ps_h[:, 0:HEADS:2, :],
                    stp1[:, 0, tk:tk + 1])
                nc.vector.tensor_scalar_mul(
                    vaugo[:, tk, :, HEAD_DIM:128], ps_h[:, 1:HEADS:2, :],
                    stp1[:, 0, tk:tk + 1])

            # ---- attention ----
            def attention(qh, hp, wq_, fin_prev=None, pv_start=4, ppk=1,
                          pop_every=1, pop_from=1, last=False):
                """S^T + exp kc-stream; PV trails pv_start kc behind; the
                previous pair's finisher (last PVs + normalization) runs at
                kc==3; thunks pop at kc>=5."""
                qsl = slice(qh * QH, (qh + 1) * QH)
                o_e = psumPV.tile([HEAD_DIM + 1, QH], f32, tag="PV",
                                  name=f"oe{qh}{hp}")
                o_o = psumPV.tile([128, QH], f32, tag="PV",
                                  name=f"oo{qh}{hp}")

                def pv(kc, a_t):
                    nc.tensor.matmul(o_e, vauge[:, kc, hp, :], a_t[:, 0:512],
                                     start=(kc == 0), stop=(kc == NT - 1))
                    nc.tensor.matmul(o_o, vaugo[:, kc, hp, :],
                                     a_t[:, 512:1024],
                                     start=(kc == 0), stop=(kc == NT - 1))

                a_ts = []
                pv_done = 0
                for kc in range(NT):
                    s_ps = psumA.tile([128, 1024], f32, tag="A")
                    ksl = slice(kc * 128, (kc + 1) * 128)
                    nc.tensor.matmul(s_ps[:, 0:512], kT[0:64, hp, ksl],
                                     qT[0:64, hp, qsl], start=True, stop=True,
                                     tile_position=(0, 0))
                    nc.tensor.matmul(s_ps[:, 512:1024], kT[64:128, hp, ksl],
                                     qT[64:128, hp, qsl], start=True,
                                     stop=True, tile_position=(64, 0))
                    a_t = attn_pool.tile([128, 1024], bf16, tag="attn")
                    nc.scalar.activation(a_t, s_ps, AF.Exp)
                    a_ts.append(a_t)
                    if kc == 0 and fin_prev is not None:
                        fin_prev()
                    if kc == 9 and pv_start == 9:
                        for k2 in range(9):
                            pv(k2, a_ts[k2])
                        pv_done = 9
                    if pv_start < 9 and kc >= pv_start and \
                            pv_done <= kc - pv_start:
                        pv(pv_done, a_ts[pv_done])
                        pv_done += 1
                    if kc >= pop_from and (kc - pop_from) % pop_every == 0:
                        for _ in range(ppk):
                            if wq_:
                                wq_.pop(0)()

                def finisher():
                    for k2 in range(pv_done, NT):
                        pv(k2, a_ts[k2])
                    # normalize: rec row, broadcast across partitions via
                    # DRAM bounce (cheap, latency hidden one pair behind) or
                    # via a PE ones-matmul for the final pair (tail latency)
                    for par, o_ps, dn in ((0, o_e, HEAD_DIM), (1, o_o, 0)):
                        den = stats.tile([1, QH], f32, tag="den", bufs=2)
                        nc.vector.tensor_copy(den, o_ps[dn:dn + 1, :])
                        recf = stats.tile([1, QH], f32, tag="recf", bufs=2)
                        nc.vector.reciprocal_approx_fast(recf, den)
                        if last:
                            bc_ps = psumM.tile([HEAD_DIM, QH], f32, tag="M")
                            nc.tensor.matmul(bc_ps, onesf, recf, start=True,
                                             stop=True)
                            bc = stats.tile([HEAD_DIM, QH], f32, tag="bcf",
                                            bufs=1)
                            nc.vector.tensor_copy(bc, bc_ps)
                        else:
                            rec = stats.tile([1, QH], bf16, tag="rec", bufs=2)
                            nc.vector.tensor_copy(rec, recf)
                            r_d = dram.tile([QH], bf16, tag="r_dram", bufs=4)
                            nc.sync.dma_start(r_d[None, :], rec)
                            bc = stats.tile([HEAD_DIM, QH], bf16, tag="bc",
                                            bufs=2)
                            bcast_load(nc.sync, bc, r_d, parts=HEAD_DIM)
                        off = 0 if par == 0 else HEAD_DIM
                        nc.vector.tensor_tensor(
                            oT[off:off + HEAD_DIM, hp, qsl],
                            o_ps[(HEAD_DIM - dn):(HEAD_DIM - dn) + HEAD_DIM,
                                 :], bc, ALU.mult)
                return finisher

            # ---- post-attention thunks ----
            mv2 = [singles.tile([128, 4, 2], f32, tag=f"mv2_{qh}",
                                name=f"mv2_{qh}") for qh in range(2)]

            def proj_chunk(qh, j, alt=False):
                tq = qh * 4 + j
                pool = psumA if alt else psumM
                ps = pool.tile([128, C], f32, tag="A" if alt else "M")
                for c in range(CC):
                    nc.tensor.matmul(ps, oT[:, c, tq * 128:(tq + 1) * 128],
                                     wp[:, c, :], start=(c == 0), stop=False)
                nc.tensor.matmul(ps, onesw, bp_row, start=False, stop=True)
                x2_t = x2[:, tq, :]
                nc.vector.tensor_tensor(x2_t, ps, xq[:, tq, :], ALU.add)
                st = stats.tile([128, 6], f32, tag="bnst")
                nc.vector.bn_stats(st, x2_t)
                nc.vector.bn_aggr(mv2[qh][:, j, :], st)

            def ln2(qh):
                rstd2 = stats.tile([128, 4], f32, tag=f"rstd2_{qh}", bufs=1,
                                   name=f"rstd2_{qh}")
                newton_rsqrt(rstd2, mv2[qh][:, :, 1], 4, f"n2{qh}")
                for j in range(4):
                    tq = qh * 4 + j
                    nc.vector.tensor_scalar(
                        z2[:, tq, :], x2[:, tq, :], mv2[qh][:, j, 0:1],
                        rstd2[:, j:j + 1], ALU.subtract, ALU.mult)

            def transz_chunk(tq, alt=False):
                pool = psumA if alt else psumM
                tp = pool.tile([128, C], bf16, tag="A" if alt else "M")
                for c in range(CC):
                    nc.tensor.transpose(
                        tp[:, c * 128:(c + 1) * 128],
                        z2[:, tq, c * 128:(c + 1) * 128], identb)
                nc.vector.tensor_copy(
                    z2T[:, :, tq * 128:(tq + 1) * 128],
                    tp.rearrange("p (c q) -> p c q", c=CC))

            def fc1_chunk(qh, m, alt=False):
                qsl = slice(qh * QH, (qh + 1) * QH)
                pool = psumA if alt else psumM
                ps = pool.tile([128, QH], f32, tag="A" if alt else "M")
                for c in range(CC):
                    nc.tensor.matmul(ps, w1[:, c, m * 128:(m + 1) * 128],
                                     z2T[:, c, qsl], start=(c == 0),
                                     stop=(c == CC - 1))
                th = work.tile([128, QH], bf16, tag="th", bufs=3)
                nc.scalar.activation(th, ps, AF.Tanh,
                                     bias=b1c[:, 1, m:m + 1], scale=GA)
                u = work.tile([128, QH], bf16, tag="u", bufs=3)
                nc.vector.tensor_scalar(u, th, 0.5, 0.5, ALU.mult, ALU.add)
                gt, mi = (gT_lo, m) if m < 6 else (gT_hi, m - 6)
                nc.vector.scalar_tensor_tensor(
                    gt[:, mi, qsl], ps, b1c[:, 0, m:m + 1], u,
                    ALU.add, ALU.mult)

            def fc2_chunk(tq, alt=False):
                pool = psumA if alt else psumM
                ps = pool.tile([128, C], f32, tag="A" if alt else "M")
                for m in range(MH):
                    gt, mi = (gT_lo, m) if m < 6 else (gT_hi, m - 6)
                    nc.tensor.matmul(ps, gt[:, mi, tq * 128:(tq + 1) * 128],
                                     w2[:, m, :], start=(m == 0), stop=False)
                nc.tensor.matmul(ps, onesw, b2_row, start=False, stop=True)
                o_t = work.tile([128, C], f32, tag="ot", bufs=2)
                nc.vector.tensor_tensor(o_t, ps, x2[:, tq, :], ALU.add)
                nc.sync.dma_start(out_d[tq * 128:(tq + 1) * 128, :], o_t)

            # ---- program ----
            psK0 = [psumA.tile([128, 1024], f32, tag="A", name=f"psK0{j}")
                    for j in range(2)]
            qk_mains(CC + 0, 0, psK0[0])
            qk_mains(CC + 0, 1, psK0[1])

            # stats bounce -> sB bcast + miv rows (all bf16)
            tp = psumM.tile([4 * NT, 128], f32, tag="M", name="st_tp")
            nc.tensor.transpose(tp, stp1.rearrange("p s k -> p (s k)"), identf)
            row = stats.tile([4 * NT, 128], bf16, tag="strow")
            nc.vector.tensor_copy(row, tp)
            sd = dram.tile([4, T], bf16, tag="st_dram")
            nc.sync.dma_start(sd.rearrange("s (k p) -> (s k) p", p=128), row)
            bcast_load(nc.sync, sB, sd[0])
            nc.sync.dma_start(miv, sd[2:4])

            qk_finish(CC + 0, 0, psK0[0])
            qk_finish(CC + 0, 1, psK0[1])
            qk_half(0, 0, 0)    # Q0 on psumM halves: the psumA double
            qk_half(0, 0, 1)    # buffer stays free for attention s_ps

            # (0,0) drains the V chunks (its deferred PV consumes them);
            # K1/Q1 halves drain in (0,1), K2/Q2 in (0,2) on psumM.
            wq1 = [lambda tk=tk: v_chunk(tk) for tk in range(1, NT)]
            wq2 = [lambda m=m, n2=n2, h2=h2: qk_half(m, n2, h2)
                   for (m, n2, h2) in ((CC + 2, 0, 0), (CC + 2, 0, 1),
                                       (CC + 2, 1, 0), (CC + 2, 1, 1),
                                       (2, 0, 0), (2, 0, 1))]
            fin = attention(0, 0, wq1, None, pv_start=9)
            fin()               # inline: V chunks arrive via own pops
            assert not wq1
            fin = attention(0, 1, wq2, None, pv_start=2, pop_every=2)
            fin = attention(0, 2, [], fin, pv_start=2)
            assert not wq2

            gT_lo = singles.tile([128, MH // 2, TQ], bf16, tag="big12",
                                 name="gT_lo")
            wq_a = [lambda j=j: proj_chunk(0, j) for j in range(4)]
            wq_a += [lambda: ln2(0)]
            wq_b = [lambda tq=tq: transz_chunk(tq) for tq in range(1, 4)]
            wq_b += [lambda m=m: fc1_chunk(0, m) for m in range(MH)]
            wq_c = [lambda tq=tq: fc2_chunk(tq) for tq in range(4)]
            fin = attention(1, 0, wq_a, fin, pv_start=2, pop_from=2,
                            pop_every=2)
            transz_chunk(0)
            fin = attention(1, 1, wq_b, fin, pv_start=2)
            fin = attention(1, 2, wq_c, fin, pv_start=2, pop_every=2,
                            pop_from=5, last=True)
            fin()

            # tail: drain leftovers, then half-1 proj/MLP
            for f in wq_a + wq_b + wq_c:
                f()
            for j in range(4):
                proj_chunk(1, j, alt=(j % 2 == 0))
            ln2(1)
            for tq in range(4, 8):
                transz_chunk(tq, alt=(tq % 2 == 0))
            for i, m in enumerate(range(MH)):
                fc1_chunk(1, m, alt=(i % 2 == 0))
            for i, tq in enumerate(range(4, 8)):
                fc2_chunk(tq, alt=(i % 2 == 0))

    nc.compile()
    return nc


def prep_inputs(x, ln1_g, ln1_b, qkv_w, qkv_b, proj_w, proj_b,
                ln2_g, ln2_b, fc1_w, fc1_b, fc2_w, fc2_b):
    """Host-side folding + per-core input maps."""
    bf16 = ml_dtypes.bfloat16
    x = np.asarray(x, np.float32)
    r = float(HEAD_DIM ** -0.25)
    qkv_w = np.asarray(qkv_w, np.float32)
    w_eff = np.asarray(ln1_g, np.float32)[:, None] * qkv_w
    b_eff = np.asarray(ln1_b, np.float32) @ qkv_w + np.asarray(qkv_b, np.float32)
    wq = w_eff[:, :C] * r
    wk = w_eff[:, C:2 * C] * r
    bq = b_eff[:C] * r
    bk = b_eff[C:2 * C] * r
    wv = w_eff[:, 2 * C:]
    bv = b_eff[2 * C:]
    wqk = np.concatenate([wq, wk], axis=1)
    sqk = wqk.sum(axis=0)
    bqk = np.concatenate([bq, bk])
    sv = wv.sum(axis=0)
    fc1_w = np.asarray(fc1_w, np.float32)
    w1_eff = np.asarray(ln2_g, np.float32)[:, None] * fc1_w
    b1_eff = np.asarray(ln2_b, np.float32) @ fc1_w + np.asarray(fc1_b, np.float32)

    def part_in(a):
        # [C_in, F] -> [128, C_in//128, F] partition-major
        ci, fo = a.shape
        return np.ascontiguousarray(
            a.reshape(ci // 128, 128, fo).transpose(1, 0, 2))

    shared = {
        "wqk": part_in(wqk).astype(bf16),
        "rqk": np.ascontiguousarray(np.stack([-sqk, bqk])).astype(bf16),
        "wv": part_in(wv).astype(bf16),
        "svbv": np.ascontiguousarray(np.stack([-sv, bv])).astype(bf16),
        "wp": part_in(np.asarray(proj_w, np.float32)).astype(bf16),
        "bp": np.asarray(proj_b, np.float32)[None, :].astype(bf16),
        "w1": part_in(w1_eff).astype(bf16),
        "b1": np.ascontiguousarray(
            np.stack([b1_eff, b1_eff * GA]).reshape(2, MH, 128)
            .transpose(2, 0, 1)).astype(np.float32),
        "w2": part_in(np.asarray(fc2_w, np.float32)).astype(bf16),
        "b2": np.asarray(fc2_b, np.float32)[None, :].astype(bf16),
    }
    in_maps = []
    for c in range(NCORES):
        b, half = c // 2, c % 2
        xb = x[b]
        xkv = np.concatenate([xb[half * TQ:(half + 1) * TQ],
                              xb[(1 - half) * TQ:(2 - half) * TQ]], axis=0)
        xq_arr = np.ascontiguousarray(
            xkv.reshape(NT, 128, C).transpose(1, 0, 2)).astype(bf16)
        xt_arr = np.ascontiguousarray(
            xkv.T.reshape(CC, 128, T).transpose(1, 0, 2)).astype(bf16)
        in_maps.append({"xq": xq_arr, "xt": xt_arr, **shared})
    return in_maps


def kernel(**inputs):
    global _COMPILED
    from concourse import bass_utils

    x = np.asarray(inputs["x"], np.float32)
    assert x.shape == (B, N, C), x.shape
    in_maps = prep_inputs(**inputs)
    if _COMPILED is None:
        _COMPILED = build_nc()
    nc = _COMPILED
    res = bass_utils.run_bass_kernel_spmd(nc, in_maps,
                                          core_ids=list(range(NCORES)))
    out = np.empty((B, N, C), np.float32)
    for c in range(NCORES):
        b, half = c // 2, c % 2
        out[b, half * TQ:(half + 1) * TQ] = res.results[c]["out"]
    return out
